# revision 3
# baseline (speedup 1.0000x reference)
"""GTE program-classification kernel v2 for 8 Trainium2 NeuronCores.

Data-parallel over dst nodes: each core handles 1024 of the 8192 dst nodes.

Key design points (vs the v1 baseline):
  - mailbox gather + the layer-0 activation transpose are precomputed on the
    host (pure indexing / layout), so the device kernel starts from plain
    contiguous DMA loads -- no indirect-DMA machinery.
  - q is pre-scaled into Wq; one fused PSUM->SBUF qkv copy per s on ACT
    (qkv laid [P, 3, S, D] so q/k/v slabs are each contiguous).
  - attention scores: per-s DVE multiply ladder in 2x bf16 mode, tail via
    TensorReduce; softmax without max-subtraction (scores are O(1)).
  - AV multiply keeps DVE 2x mode via a x2-packed pn replica (pn2) with
    3-free-dim APs; the t-reduction runs on the PE as an identity-matmul
    PSUM accumulation (8 x 512-col matmuls per s) instead of DVE adds.
  - residual adds fused with the PSUM eviction via scalar_tensor_tensor,
    whose accum_out yields the LN mean for free; LN sum-of-squares on ACT
    (Square+accum), normalize via 4x TensorScalarPtr.
  - 2-stage software pipeline across node tiles: tile i+1's layer-0 unit
    runs interleaved with tile i's layer-1 unit; QKV of the next units is
    prefetched into the attention windows; the FFN2 residual is deferred
    past the next attention block via an SBUF staging tile (fb).
GPSIMD is not used for compute (walrus rejects generic ops on Pool).
"""
import sys
if '/opt/trn_rl_repo' not in sys.path:
    sys.path.insert(0, '/opt/trn_rl_repo')

import numpy as np
import ml_dtypes

import concourse.bass as bass
import concourse.tile as tile
import concourse.mybir as mybir
from concourse.bass import ds
from concourse.bass_utils import run_bass_kernel_spmd

F32 = mybir.dt.float32
BF16 = mybir.dt.bfloat16
I32 = mybir.dt.int32
AF = mybir.ActivationFunctionType
OP = mybir.AluOpType
AX = mybir.AxisListType

P = 128
D = 512
H = 8
DH = 64
S = 8          # messages used per node (9th dropped by the reference)
NL = 2
V = 50000
NCLS = 104
DFF = 1024
NDST = 8192
NSRC = 40000
NCORES = 8
NLOC = NDST // NCORES      # 1024 dst nodes per core
NT = NLOC // P             # 8 node tiles per core
DC = D // P                # 4 d-chunks
FCH = DFF // P             # 8 dff-chunks
LN_EPS = 1e-5

NORM_ACT = False    # LN normalize on ACT instead of DVE TensorScalarPtr
PN2_ACT = True      # pn2 replica on ACT (else DVE)


def _split_multiwait_drains(nc):
    """walrus in this container accepts only one sync-wait per instruction;
    split any multi-wait Drain into a chain of single-wait drains."""
    for fn in nc.m.functions:
        for bb in fn.blocks:
            newlist = []
            for ins in bb.instructions:
                si = ins.sync_info
                if si is not None and si.on_wait and len(si.on_wait) > 1:
                    waits = list(si.on_wait)
                    for j, w in enumerate(waits[:-1]):
                        d = mybir.InstDrain(name=f'{ins.name}-sw{j}',
                                            engine=ins.engine)
                        d.sync_info = mybir.SyncInfo(on_wait=[w], on_update=[])
                        newlist.append(d)
                    si.on_wait = [waits[-1]]
                newlist.append(ins)
            bb.instructions[:] = newlist


def build(flags):
    nc = bass.Bass()

    xg_d = nc.dram_tensor("xg", [NLOC, S * D], BF16, kind="ExternalInput")
    xt0_d = nc.dram_tensor("xt0", [NT, DC, P, S * P], BF16,
                           kind="ExternalInput")
    wq_d = nc.dram_tensor("wqkvT", [NL, D, 3 * D], BF16, kind="ExternalInput")
    wo_d = nc.dram_tensor("woT", [NL, D, D], BF16, kind="ExternalInput")
    w1_d = nc.dram_tensor("w1T", [NL, D, DFF], BF16, kind="ExternalInput")
    w2_d = nc.dram_tensor("w2T", [NL, DFF, D], BF16, kind="ExternalInput")
    wf_d = nc.dram_tensor("wfcT", [D, NCLS], BF16, kind="ExternalInput")
    id_d = nc.dram_tensor("ident", [P, P], BF16, kind="ExternalInput")
    out_d = nc.dram_tensor("logits", [NLOC, NCLS], F32, kind="ExternalOutput")

    need_vec = {}
    if flags['bqkv']:
        need_vec['bqkv'] = [NL, 3 * D]
    if flags['bo']:
        need_vec['bo'] = [NL, D]
    if flags['b2']:
        need_vec['b2'] = [NL, D]
    if flags['bfc']:
        need_vec['bfc'] = [1, NCLS]
    if flags['ln_g']:
        need_vec['ln1_g'] = [NL, D]
        need_vec['ln2_g'] = [NL, D]
    if flags['ln_b']:
        need_vec['ln1_b'] = [NL, D]
        need_vec['ln2_b'] = [NL, D]
    # replicated vectors: host passes them pre-broadcast as [P, n]
    vec_d = {k: nc.dram_tensor(k, [P, shp[0] * shp[1]], F32,
                               kind="ExternalInput")
             for k, shp in need_vec.items()}
    b1t_d = (nc.dram_tensor("b1t", [P, NL * FCH], F32, kind="ExternalInput")
             if flags['b1'] else None)

    with tile.TileContext(nc) as tc:
        with tc.tile_pool(name="wpool", bufs=1) as wp, \
             tc.tile_pool(name="dbl", bufs=2) as dbl, \
             tc.tile_pool(name="dbl3", bufs=3) as dbl3, \
             tc.tile_pool(name="sp", bufs=2) as sp, \
             tc.tile_pool(name="psA", bufs=2, space="PSUM") as psA, \
             tc.tile_pool(name="psB", bufs=2, space="PSUM") as psB:

            # ---- resident weights (bf16) ----
            wq_sb, wo_sb, w1_sb, w2_sb = [], [], [], []
            for l in range(NL):
                t = wp.tile([P, DC, 3 * D], BF16, tag=f"wq{l}")
                for c in range(DC):
                    nc.sync.dma_start(t[:, c, :], wq_d[l, c * P:(c + 1) * P, :])
                wq_sb.append(t)
                t = wp.tile([P, DC, D], BF16, tag=f"wo{l}")
                for c in range(DC):
                    nc.sync.dma_start(t[:, c, :], wo_d[l, c * P:(c + 1) * P, :])
                wo_sb.append(t)
                t = wp.tile([P, DC, DFF], BF16, tag=f"w1{l}")
                for c in range(DC):
                    nc.sync.dma_start(t[:, c, :], w1_d[l, c * P:(c + 1) * P, :])
                w1_sb.append(t)
                t = wp.tile([P, FCH, D], BF16, tag=f"w2{l}")
                for c in range(FCH):
                    nc.sync.dma_start(t[:, c, :], w2_d[l, c * P:(c + 1) * P, :])
                w2_sb.append(t)
            wf_sb = wp.tile([P, DC, NCLS], BF16, tag="wf")
            for c in range(DC):
                nc.sync.dma_start(wf_sb[:, c, :], wf_d[c * P:(c + 1) * P, :])
            id_sb = wp.tile([P, P], BF16, tag="ident")
            nc.sync.dma_start(id_sb[:], id_d[:])

            vec_sb = {}
            for k in need_vec:
                tb = wp.tile([P, vec_d[k].shape[1]], F32, tag=f"{k}_rep")
                nc.sync.dma_start(tb[:], vec_d[k][:])
                vec_sb[k] = tb

            b1t_sb = None
            if flags['b1']:
                b1t_sb = wp.tile([P, NL * FCH], F32, tag="b1t")
                nc.sync.dma_start(b1t_sb[:], b1t_d[:])

            def vsl(k, l, n):
                return vec_sb[k][:, l * n:(l + 1) * n]

            def layernorm(x, sums, l, gk, bk):
                """x: [P,S,D] bf16 post-residual; sums: [P,S] f32 row-sums.
                In-place normalize; sum-of-squares on ACT."""
                sq = wp.tile([P, D], BF16, tag="lnsq")
                st = wp.tile([P, 4 * S], F32, tag="lnstat")
                qs = st[:, 0:S]
                nmean = st[:, S:2 * S]
                var = st[:, 2 * S:3 * S]
                rstd = st[:, 3 * S:4 * S]
                for s in range(S):
                    nc.scalar.activation(sq[:], x[:, s, :], AF.Square,
                                         accum_out=qs[:, s:s + 1])
                nc.vector.tensor_scalar_mul(nmean[:], sums[:], -1.0 / D)
                nc.vector.tensor_tensor(out=var[:], in0=nmean[:], in1=nmean[:],
                                        op=OP.mult)
                nc.vector.scalar_tensor_tensor(
                    out=var[:], in0=qs[:], scalar=1.0 / D, in1=var[:],
                    op0=OP.mult, op1=OP.subtract)
                nc.vector.tensor_scalar_add(var[:], var[:], LN_EPS)
                nc.vector.reciprocal(var[:], var[:])
                nc.scalar.activation(rstd[:], var[:], AF.Sqrt)
                for s in range(S):
                    nc.vector.tensor_scalar(
                        out=x[:, s, :], in0=x[:, s, :],
                        scalar1=nmean[:, s:s + 1],
                        scalar2=rstd[:, s:s + 1],
                        op0=OP.add, op1=OP.mult)
                if gk is not None:
                    for s in range(S):
                        nc.vector.tensor_tensor(out=x[:, s, :], in0=x[:, s, :],
                                                in1=gk, op=OP.mult)
                if bk is not None:
                    for s in range(S):
                        nc.vector.tensor_tensor(out=x[:, s, :], in0=x[:, s, :],
                                                in1=bk, op=OP.add)

            def emit_gather(i, st):
                x = dbl3.tile([P, S, D], BF16, tag="x")
                nc.sync.dma_start(
                    x[:].rearrange("p s d -> p (s d)"),
                    xg_d[ds(i * P, P), :])
                st[i] = {'x': x}

            def emit_A_mm(i, l, st):
                """transposes + QKV matmuls for (i, l)."""
                x = st[i]['x']
                xT = [dbl3.tile([P, DC, 4 * P], BF16, tag="xT",
                                name=f"xT{h}") for h in range(2)]
                if l == 0:
                    for h in range(2):
                        nc.sync.dma_start(
                            xT[h][:],
                            xt0_d[i, :, :, h * 4 * P:(h + 1) * 4 * P]
                            .transpose([1, 0, 2]))
                else:
                    for s in range(S):
                        for c in range(DC):
                            nc.sync.dma_start_transpose(
                                xT[s // 4][:, c, (s % 4) * P:(s % 4 + 1) * P],
                                x[:, s, c * P:(c + 1) * P])
                pqs = []
                for s in range(S):
                    pq = psA.tile([P, 3 * D], F32, tag="pqkv")
                    for c in range(DC):
                        lhsT = xT[s // 4][:, c, (s % 4) * P:(s % 4 + 1) * P]
                        for nb in range(3):
                            nc.tensor.matmul(
                                pq[:, nb * D:(nb + 1) * D], lhsT,
                                wq_sb[l][:, c, nb * D:(nb + 1) * D],
                                start=(c == 0), stop=(c == DC - 1))
                    pqs.append(pq)
                st[i][f'pq{l}'] = pqs

            def emit_A_cp(i, l, st):
                # qkv laid [P, 3, S, D]: q/k/v slabs each contiguous
                qkv = dbl.tile([P, 3, S, D], BF16, tag="qkv")
                for s, pq in enumerate(st[i][f'pq{l}']):
                    if flags['bqkv']:
                        nc.vector.tensor_add(pq[:], pq[:],
                                             vsl('bqkv', l, 3 * D))
                    nc.scalar.copy(
                        qkv[:, :, s, :],
                        pq[:].rearrange("p (n d) -> p n d", n=3))
                st[i]['qkv'] = qkv

            def emit_A(i, l, st):
                emit_A_mm(i, l, st)
                emit_A_cp(i, l, st)

            def emit_Bq(i, l, st):
                """attention scores + softmax + pn2 for (i, l)."""
                qkv = st[i]['qkv']
                scores = wp.tile([P, S, H, S], BF16, tag="scores")
                for s in range(S):
                    qk = dbl.tile([P, S, H, DH], BF16, tag="qks")
                    nc.vector.tensor_tensor(
                        out=qk[:],
                        in0=qkv[:, 1, :, :]
                            .rearrange("p t (h e) -> p t h e", h=H),
                        in1=qkv[:, 0, s, :]
                            .rearrange("p (h e) -> p h e", h=H)
                            .unsqueeze(1).broadcast_to([P, S, H, DH]),
                        op=OP.mult)
                    for w in (32, 16, 8):
                        nc.vector.tensor_tensor(out=qk[:, :, :, 0:w],
                                                in0=qk[:, :, :, 0:w],
                                                in1=qk[:, :, :, w:2 * w],
                                                op=OP.add)
                    with nc.allow_low_precision("bf16 scores ok at 2e-2"):
                        nc.vector.reduce_sum(
                            scores[:, s, :, :].transpose([0, 2, 1]),
                            qk[:, :, :, 0:8], axis=AX.X)

                # softmax over t (scores O(1): skip max-sub); exp in place
                pnf = wp.tile([P, S * H], F32, tag="den")
                nc.scalar.activation(
                    scores[:].rearrange("p s h t -> p (s h t)"),
                    scores[:].rearrange("p s h t -> p (s h t)"), AF.Exp)
                den = pnf[:].rearrange("p (s h) -> p s h", s=S)
                nc.vector.reduce_sum(den, scores[:], axis=AX.X)
                nc.vector.reciprocal(pnf[:], pnf[:])
                # pn laid [P, s, t, h] (strided write, (s,h,t) iteration)
                pn = wp.tile([P, S, S, H], BF16, tag="pn")
                nc.vector.tensor_tensor(
                    out=pn[:].transpose([0, 1, 3, 2]), in0=scores[:],
                    in1=den.unsqueeze(3).broadcast_to([P, S, H, S]),
                    op=OP.mult)
                # x2-packed replica [P, s, t, h, 2], written per-s (3D APs)
                pn2 = sp.tile([P, S, S, H, 2], BF16, tag="pn2")
                for s in range(S):
                    if PN2_ACT:
                        nc.scalar.copy(
                            pn2[:, s, :, :, :],
                            pn[:, s, :, :].unsqueeze(3)
                            .broadcast_to([P, S, H, 2]))
                    else:
                        nc.vector.tensor_copy(
                            pn2[:, s, :, :, :],
                            pn[:, s, :, :].unsqueeze(3)
                            .broadcast_to([P, S, H, 2]))
                st[i]['pn2'] = pn2

            def emit_Bv(i, l, st):
                """AV + Wo + LN1 + FFN1 + FFN2-matmuls for (i, l)."""
                x = st[i]['x']
                qkv = st[i]['qkv']
                pn2 = st[i]['pn2']
                aT = [dbl.tile([P, DC, 4 * P], BF16, tag="aT2",
                               name=f"aTh{h}") for h in range(2)]
                sums1 = wp.tile([P, S], F32, tag="sums1")
                pos = [None] * S

                def resid1(s):
                    nc.vector.scalar_tensor_tensor(
                        out=x[:, s, :], in0=pos[s][:], scalar=1.0,
                        in1=x[:, s, :], op0=OP.mult, op1=OP.add,
                        accum_out=sums1[:, s:s + 1])

                for s in range(S):
                    av = dbl.tile([P, S, H, DH], BF16, tag="qks")
                    # 3-free-dim APs: (t h) merged, e split as 32x2
                    pn_in = pn2[:, s, :, :, :] \
                        .rearrange("p t h b -> p (t h) b") \
                        .unsqueeze(2).broadcast_to([P, S * H, 32, 2])
                    nc.vector.tensor_tensor(
                        out=av[:].rearrange("p t h (a b) -> p (t h) a b",
                                            b=2),
                        in0=qkv[:, 2, :, :]
                            .rearrange("p t (h a b) -> p (t h) a b",
                                       h=H, b=2),
                        in1=pn_in,
                        op=OP.mult)
                    # t-reduction on PE: identity-matmul PSUM accumulation
                    pa = psB.tile([P, D], F32, tag="mm")
                    for t in range(S):
                        nc.tensor.matmul(
                            pa[:], id_sb[:],
                            av[:, t, :, :].rearrange("p h e -> p (h e)"),
                            start=(t == 0), stop=(t == S - 1))
                    asb = wp.tile([P, D], BF16, tag="asb")
                    nc.scalar.copy(asb[:], pa[:])
                    for c in range(DC):
                        nc.sync.dma_start_transpose(
                            aT[s // 4][:, c, (s % 4) * P:(s % 4 + 1) * P],
                            asb[:, c * P:(c + 1) * P])
                    po = psB.tile([P, D], F32, tag="mm")
                    for c in range(DC):
                        nc.tensor.matmul(
                            po[:],
                            aT[s // 4][:, c, (s % 4) * P:(s % 4 + 1) * P],
                            wo_sb[l][:, c, :],
                            start=(c == 0), stop=(c == DC - 1))
                    if flags['bo']:
                        nc.vector.tensor_add(po[:], po[:], vsl('bo', l, D))
                    pos[s] = po
                    if s >= 2:
                        resid1(s - 2)
                resid1(S - 2)
                resid1(S - 1)

                layernorm(x, sums1, l,
                          vsl('ln1_g', l, D) if flags['ln_g'] else None,
                          vsl('ln1_b', l, D) if flags['ln_b'] else None)

                # x1^T for FFN1
                x1T = [dbl.tile([P, DC, 4 * P], BF16, tag="aT2",
                                name=f"x1Th{h}") for h in range(2)]
                for s in range(S):
                    for c in range(DC):
                        nc.sync.dma_start_transpose(
                            x1T[s // 4][:, c, (s % 4) * P:(s % 4 + 1) * P],
                            x[:, s, c * P:(c + 1) * P])

                # FFN1 -> h^T [dff-part, tok], fused relu (+b1)
                hT = wp.tile([P, FCH, S * P], BF16, tag="hT")
                for m in range(FCH):
                    for hf in range(2):
                        ph = psB.tile([P, D], F32, tag="mm")
                        for c in range(DC):
                            nc.tensor.matmul(
                                ph[:],
                                w1_sb[l][:, c, m * P:(m + 1) * P],
                                x1T[hf][:, c, :],
                                start=(c == 0), stop=(c == DC - 1))
                        if flags['b1']:
                            nc.scalar.activation(
                                hT[:, m, hf * D:(hf + 1) * D], ph[:],
                                AF.Relu,
                                bias=b1t_sb[:, l * FCH + m:l * FCH + m + 1])
                        else:
                            nc.scalar.activation(
                                hT[:, m, hf * D:(hf + 1) * D], ph[:],
                                AF.Relu)

                # FFN2 matmuls; results staged to SBUF (fb) by ACT so the
                # DVE-side residual can run after the next attention block
                fb = wp.tile([P, S, D], BF16, tag="fb")
                for s in range(S):
                    pf = psB.tile([P, D], F32, tag="mm")
                    for k in range(FCH):
                        nc.tensor.matmul(pf[:],
                                         hT[:, k, s * P:(s + 1) * P],
                                         w2_sb[l][:, k, :],
                                         start=(k == 0),
                                         stop=(k == FCH - 1))
                    if flags['b2']:
                        nc.vector.tensor_add(pf[:], pf[:], vsl('b2', l, D))
                    nc.scalar.copy(fb[:, s, :], pf[:])
                st[i][f'fb{l}'] = fb

            def emit_C(i, l, st):
                """FFN2 residual (from staged fb) + LN2."""
                x = st[i]['x']
                fb = st[i][f'fb{l}']
                sums2 = wp.tile([P, S], F32, tag="sums2")
                for s in range(S):
                    nc.vector.scalar_tensor_tensor(
                        out=x[:, s, :], in0=fb[:, s, :], scalar=1.0,
                        in1=x[:, s, :], op0=OP.mult, op1=OP.add,
                        accum_out=sums2[:, s:s + 1])
                layernorm(x, sums2, l,
                          vsl('ln2_g', l, D) if flags['ln_g'] else None,
                          vsl('ln2_b', l, D) if flags['ln_b'] else None)

            def emit_epi(i, st):
                x = st[i]['x']
                nc.vector.tensor_tensor(out=x[:, 0:4, :], in0=x[:, 0:4, :],
                                        in1=x[:, 4:8, :], op=OP.max)
                nc.vector.tensor_tensor(out=x[:, 0:2, :], in0=x[:, 0:2, :],
                                        in1=x[:, 2:4, :], op=OP.max)
                rst = wp.tile([P, D], BF16, tag="rst")
                nc.vector.tensor_tensor(out=rst[:], in0=x[:, 0, :],
                                        in1=x[:, 1, :], op=OP.max)
                rT = wp.tile([P, DC, P], BF16, tag="asb")
                for c in range(DC):
                    nc.sync.dma_start_transpose(rT[:, c, :],
                                                rst[:, c * P:(c + 1) * P])
                pc = psB.tile([P, D], F32, tag="mm")
                for c in range(DC):
                    nc.tensor.matmul(pc[:, 0:NCLS], rT[:, c, :], wf_sb[:, c, :],
                                     start=(c == 0), stop=(c == DC - 1))
                if flags['bfc']:
                    nc.vector.tensor_add(pc[:, 0:NCLS], pc[:, 0:NCLS],
                                         vec_sb['bfc'][:, :])
                lg = wp.tile([P, NCLS], F32, tag="lg")
                nc.scalar.copy(lg[:], pc[:, 0:NCLS])
                nc.sync.dma_start(out_d[ds(i * P, P), :], lg[:])

            # ---- software-pipelined emission ----
            st = {}
            emit_gather(0, st)
            emit_A(0, 0, st)
            if NT > 1:
                emit_gather(1, st)
                emit_A(1, 0, st)
            emit_Bq(0, 0, st)
            emit_Bv(0, 0, st)
            emit_C(0, 0, st)
            for i in range(NT):
                emit_A(i, 1, st)
                if i + 2 < NT:
                    emit_gather(i + 2, st)
                    emit_A_mm(i + 2, 0, st)
                if i + 1 < NT:
                    emit_Bq(i + 1, 0, st)
                    emit_Bv(i + 1, 0, st)
                emit_Bq(i, 1, st)
                emit_Bv(i, 1, st)
                if i + 2 < NT:
                    emit_A_cp(i + 2, 0, st)
                if i + 1 < NT:
                    emit_C(i + 1, 0, st)
                emit_C(i, 1, st)
                emit_epi(i, st)

    _split_multiwait_drains(nc)
    return nc


OPT_KEYS = ('bqkv', 'bo', 'b1', 'b2', 'bfc', 'ln_g', 'ln_b')
_cache = {}


def _get_nc(flags):
    key = tuple(flags[k] for k in OPT_KEYS)
    if key not in _cache:
        _cache[key] = build(flags)
    return _cache[key]


def _prep_inputs(inputs, flags):
    bf = ml_dtypes.bfloat16
    token_ids = np.asarray(inputs['token_ids'])
    edge_src = np.asarray(inputs['edge_src'])
    emb = np.asarray(inputs['emb'], dtype=np.float32)
    Wqkv = np.asarray(inputs['Wqkv'], dtype=np.float32)
    Wo = np.asarray(inputs['Wo'], dtype=np.float32)
    W1 = np.asarray(inputs['W1'], dtype=np.float32)
    W2 = np.asarray(inputs['W2'], dtype=np.float32)
    Wfc = np.asarray(inputs['Wfc'], dtype=np.float32)

    tid2 = token_ids[edge_src[:, :S]]                      # [NDST, S]
    xg = emb.astype(bf)[tid2]                              # [NDST, S, D]
    wqkvT = np.ascontiguousarray(Wqkv.transpose(0, 2, 1))  # [NL, D, 3D]
    wqkvT[:, :, 0:D] *= 0.125                              # fold q scale
    common = {
        'wqkvT': wqkvT.astype(bf),
        'woT': np.ascontiguousarray(Wo.transpose(0, 2, 1)).astype(bf),
        'w1T': np.ascontiguousarray(W1.transpose(0, 2, 1)).astype(bf),
        'w2T': np.ascontiguousarray(W2.transpose(0, 2, 1)).astype(bf),
        'wfcT': np.ascontiguousarray(Wfc.T).astype(bf),
        'ident': np.eye(P, dtype=bf),
    }
    if flags['bqkv']:
        bq = np.asarray(inputs['bqkv'], dtype=np.float32).copy()
        bq[:, 0:D] *= 0.125
        common['bqkv'] = np.ascontiguousarray(
            np.broadcast_to(bq.reshape(1, -1), (P, NL * 3 * D)))
    if flags['bo']:
        common['bo'] = np.ascontiguousarray(np.broadcast_to(
            np.asarray(inputs['bo'], np.float32).reshape(1, -1),
            (P, NL * D)))
    if flags['b1']:
        b1 = np.asarray(inputs['b1'], dtype=np.float32)
        common['b1t'] = np.ascontiguousarray(
            b1.reshape(NL, FCH, P).transpose(2, 0, 1).reshape(P, NL * FCH))
    if flags['b2']:
        common['b2'] = np.ascontiguousarray(np.broadcast_to(
            np.asarray(inputs['b2'], np.float32).reshape(1, -1),
            (P, NL * D)))
    if flags['bfc']:
        common['bfc'] = np.ascontiguousarray(np.broadcast_to(
            np.asarray(inputs['bfc'], np.float32).reshape(1, -1),
            (P, NCLS)))
    if flags['ln_g']:
        common['ln1_g'] = np.ascontiguousarray(np.broadcast_to(
            np.asarray(inputs['ln1_g'], np.float32).reshape(1, -1),
            (P, NL * D)))
        common['ln2_g'] = np.ascontiguousarray(np.broadcast_to(
            np.asarray(inputs['ln2_g'], np.float32).reshape(1, -1),
            (P, NL * D)))
    if flags['ln_b']:
        common['ln1_b'] = np.ascontiguousarray(np.broadcast_to(
            np.asarray(inputs['ln1_b'], np.float32).reshape(1, -1),
            (P, NL * D)))
        common['ln2_b'] = np.ascontiguousarray(np.broadcast_to(
            np.asarray(inputs['ln2_b'], np.float32).reshape(1, -1),
            (P, NL * D)))

    in_maps = []
    for c in range(NCORES):
        m = dict(common)
        xc = xg[c * NLOC:(c + 1) * NLOC]                   # [NLOC, S, D]
        m['xg'] = np.ascontiguousarray(xc.reshape(NLOC, S * D))
        # xt0[i, c, p, s*128+n] = x[i*128+n, s, c*128+p]
        m['xt0'] = np.ascontiguousarray(
            xc.reshape(NT, P, S, DC, P).transpose(0, 3, 4, 2, 1)
            .reshape(NT, DC, P, S * P))
        in_maps.append(m)
    return in_maps


def _get_flags(inputs):
    return {
        'bqkv': bool(np.any(inputs['bqkv'])),
        'bo': bool(np.any(inputs['bo'])),
        'b1': bool(np.any(inputs['b1'])),
        'b2': bool(np.any(inputs['b2'])),
        'bfc': bool(np.any(inputs['bfc'])),
        'ln_g': bool(np.any(np.asarray(inputs['ln1_g']) != 1.0)
                     or np.any(np.asarray(inputs['ln2_g']) != 1.0)),
        'ln_b': bool(np.any(inputs['ln1_b']) or np.any(inputs['ln2_b'])),
    }


def kernel(**inputs):
    flags = _get_flags(inputs)
    nc = _get_nc(flags)
    in_maps = _prep_inputs(inputs, flags)
    res = run_bass_kernel_spmd(nc, in_maps, core_ids=list(range(NCORES)))
    out = np.concatenate([res.results[c]['logits'] for c in range(NCORES)],
                         axis=0)
    return out.astype(np.float32)


if __name__ == '__main__':
    import time
    sys.path.insert(0, '/root/problem')
    import reference
    inp = {k: np.asarray(v) for k, v in reference.setup_inputs().items()}
    t0 = time.time()
    got = kernel(**inp)
    print(f"kernel ran in {time.time()-t0:.1f}s")
    exp = np.asarray(reference.reference(**reference.setup_inputs()))
    err = np.abs(got - exp).max()
    rel = err / np.abs(exp).max()
    print(f"absmax err {err:.3e}  rel {rel:.3e}")


# revision 7
# speedup vs baseline: 4.3858x; 4.3858x over previous
"""GTE program-classification kernel v2 for 8 Trainium2 NeuronCores.

Data-parallel over dst nodes: each core handles 1024 of the 8192 dst nodes.

Key design points (vs the v1 baseline):
  - mailbox gather + the layer-0 activation transpose are precomputed on the
    host (pure indexing / layout), so the device kernel starts from plain
    contiguous DMA loads -- no indirect-DMA machinery.
  - q is pre-scaled into Wq; one fused PSUM->SBUF qkv copy per s on ACT
    (qkv laid [P, 3, S, D] so q/k/v slabs are each contiguous).
  - attention scores: per-s DVE multiply ladder in 2x bf16 mode, tail via
    TensorReduce; softmax without max-subtraction (scores are O(1)).
  - AV multiply keeps DVE 2x mode via a x2-packed pn replica (pn2) with
    3-free-dim APs; the t-reduction runs on the PE as an identity-matmul
    PSUM accumulation (8 x 512-col matmuls per s) instead of DVE adds.
  - residual adds fused with the PSUM eviction via scalar_tensor_tensor,
    whose accum_out yields the LN mean for free; LN sum-of-squares on ACT
    (Square+accum), normalize via 4x TensorScalarPtr.
  - 2-stage software pipeline across node tiles: tile i+1's layer-0 unit
    runs interleaved with tile i's layer-1 unit; QKV of the next units is
    prefetched into the attention windows; the FFN2 residual is deferred
    past the next attention block via an SBUF staging tile (fb).
GPSIMD is not used for compute (walrus rejects generic ops on Pool).
"""
import sys
if '/opt/trn_rl_repo' not in sys.path:
    sys.path.insert(0, '/opt/trn_rl_repo')

import numpy as np
import ml_dtypes

import concourse.bass as bass
import concourse.tile as tile
import concourse.mybir as mybir
from concourse.bass import ds
from concourse.bass_utils import run_bass_kernel_spmd

F32 = mybir.dt.float32
BF16 = mybir.dt.bfloat16
I32 = mybir.dt.int32
AF = mybir.ActivationFunctionType
OP = mybir.AluOpType
AX = mybir.AxisListType

P = 128
D = 512
H = 8
DH = 64
S = 8          # messages used per node (9th dropped by the reference)
NL = 2
V = 50000
NCLS = 104
DFF = 1024
NDST = 8192
NSRC = 40000
NCORES = 8
NLOC = NDST // NCORES      # 1024 dst nodes per core
NT = NLOC // P             # 8 node tiles per core
DC = D // P                # 4 d-chunks
FCH = DFF // P             # 8 dff-chunks
LN_EPS = 1e-5

NORM_ACT = False    # LN normalize on ACT instead of DVE TensorScalarPtr
PN2_ACT = True      # pn2 replica on ACT (else DVE)


def _split_multiwait_drains(nc):
    """walrus in this container accepts only one sync-wait per instruction;
    split any multi-wait Drain into a chain of single-wait drains."""
    for fn in nc.m.functions:
        for bb in fn.blocks:
            newlist = []
            for ins in bb.instructions:
                si = ins.sync_info
                if si is not None and si.on_wait and len(si.on_wait) > 1:
                    waits = list(si.on_wait)
                    for j, w in enumerate(waits[:-1]):
                        d = mybir.InstDrain(name=f'{ins.name}-sw{j}',
                                            engine=ins.engine)
                        d.sync_info = mybir.SyncInfo(on_wait=[w], on_update=[])
                        newlist.append(d)
                    si.on_wait = [waits[-1]]
                newlist.append(ins)
            bb.instructions[:] = newlist


def build(flags):
    nc = bass.Bass()

    xg_d = nc.dram_tensor("xg", [NLOC, S * D], BF16, kind="ExternalInput")
    xt0_d = nc.dram_tensor("xt0", [NT, DC, P, S * P], BF16,
                           kind="ExternalInput")
    wq_d = nc.dram_tensor("wqkvT", [NL, D, 3 * D], BF16, kind="ExternalInput")
    wo_d = nc.dram_tensor("woT", [NL, D, D], BF16, kind="ExternalInput")
    w1_d = nc.dram_tensor("w1T", [NL, D, DFF], BF16, kind="ExternalInput")
    w2_d = nc.dram_tensor("w2T", [NL, DFF, D], BF16, kind="ExternalInput")
    wf_d = nc.dram_tensor("wfcT", [D, NCLS], BF16, kind="ExternalInput")
    id_d = nc.dram_tensor("ident", [P, P], BF16, kind="ExternalInput")
    out_d = nc.dram_tensor("logits", [NLOC, NCLS], F32, kind="ExternalOutput")

    need_vec = {}
    if flags['bqkv']:
        need_vec['bqkv'] = [NL, 3 * D]
    if flags['bo']:
        need_vec['bo'] = [NL, D]
    if flags['b2']:
        need_vec['b2'] = [NL, D]
    if flags['bfc']:
        need_vec['bfc'] = [1, NCLS]
    if flags['ln_g']:
        need_vec['ln1_g'] = [NL, D]
        need_vec['ln2_g'] = [NL, D]
    if flags['ln_b']:
        need_vec['ln1_b'] = [NL, D]
        need_vec['ln2_b'] = [NL, D]
    # replicated vectors: host passes them pre-broadcast as [P, n]
    vec_d = {k: nc.dram_tensor(k, [P, shp[0] * shp[1]], F32,
                               kind="ExternalInput")
             for k, shp in need_vec.items()}
    b1t_d = (nc.dram_tensor("b1t", [P, NL * FCH], F32, kind="ExternalInput")
             if flags['b1'] else None)

    with tile.TileContext(nc) as tc:
        with tc.tile_pool(name="wpool", bufs=1) as wp, \
             tc.tile_pool(name="dbl", bufs=2) as dbl, \
             tc.tile_pool(name="dbl3", bufs=3) as dbl3, \
             tc.tile_pool(name="sp", bufs=2) as sp, \
             tc.tile_pool(name="psA", bufs=2, space="PSUM") as psA, \
             tc.tile_pool(name="psB", bufs=2, space="PSUM") as psB:

            # ---- resident weights (bf16) ----
            wq_sb, wo_sb, w1_sb, w2_sb = [], [], [], []
            for l in range(NL):
                t = wp.tile([P, DC, 3 * D], BF16, tag=f"wq{l}")
                for c in range(DC):
                    nc.sync.dma_start(t[:, c, :], wq_d[l, c * P:(c + 1) * P, :])
                wq_sb.append(t)
                t = wp.tile([P, DC, D], BF16, tag=f"wo{l}")
                for c in range(DC):
                    nc.sync.dma_start(t[:, c, :], wo_d[l, c * P:(c + 1) * P, :])
                wo_sb.append(t)
                t = wp.tile([P, DC, DFF], BF16, tag=f"w1{l}")
                for c in range(DC):
                    nc.sync.dma_start(t[:, c, :], w1_d[l, c * P:(c + 1) * P, :])
                w1_sb.append(t)
                t = wp.tile([P, FCH, D], BF16, tag=f"w2{l}")
                for c in range(FCH):
                    nc.sync.dma_start(t[:, c, :], w2_d[l, c * P:(c + 1) * P, :])
                w2_sb.append(t)
            wf_sb = wp.tile([P, DC, NCLS], BF16, tag="wf")
            for c in range(DC):
                nc.sync.dma_start(wf_sb[:, c, :], wf_d[c * P:(c + 1) * P, :])
            id_sb = wp.tile([P, P], BF16, tag="ident")
            nc.sync.dma_start(id_sb[:], id_d[:])

            vec_sb = {}
            for k in need_vec:
                tb = wp.tile([P, vec_d[k].shape[1]], F32, tag=f"{k}_rep")
                nc.sync.dma_start(tb[:], vec_d[k][:])
                vec_sb[k] = tb

            b1t_sb = None
            if flags['b1']:
                b1t_sb = wp.tile([P, NL * FCH], F32, tag="b1t")
                nc.sync.dma_start(b1t_sb[:], b1t_d[:])

            def vsl(k, l, n):
                return vec_sb[k][:, l * n:(l + 1) * n]

            def layernorm(x, sums, l, gk, bk):
                """x: [P,S,D] bf16 post-residual; sums: [P,S] f32 row-sums.
                In-place normalize; sum-of-squares on ACT."""
                sq = wp.tile([P, D], BF16, tag="lnsq")
                st = wp.tile([P, 4 * S], F32, tag="lnstat")
                qs = st[:, 0:S]
                nmean = st[:, S:2 * S]
                var = st[:, 2 * S:3 * S]
                rstd = st[:, 3 * S:4 * S]
                for s in range(S):
                    nc.scalar.activation(sq[:], x[:, s, :], AF.Square,
                                         accum_out=qs[:, s:s + 1])
                nc.vector.tensor_scalar_mul(nmean[:], sums[:], -1.0 / D)
                nc.vector.tensor_tensor(out=var[:], in0=nmean[:], in1=nmean[:],
                                        op=OP.mult)
                nc.vector.scalar_tensor_tensor(
                    out=var[:], in0=qs[:], scalar=1.0 / D, in1=var[:],
                    op0=OP.mult, op1=OP.subtract)
                nc.vector.tensor_scalar_add(var[:], var[:], LN_EPS)
                nc.vector.reciprocal(var[:], var[:])
                nc.scalar.activation(rstd[:], var[:], AF.Sqrt)
                for s in range(S):
                    nc.vector.tensor_scalar(
                        out=x[:, s, :], in0=x[:, s, :],
                        scalar1=nmean[:, s:s + 1],
                        scalar2=rstd[:, s:s + 1],
                        op0=OP.add, op1=OP.mult)
                if gk is not None:
                    for s in range(S):
                        nc.vector.tensor_tensor(out=x[:, s, :], in0=x[:, s, :],
                                                in1=gk, op=OP.mult)
                if bk is not None:
                    for s in range(S):
                        nc.vector.tensor_tensor(out=x[:, s, :], in0=x[:, s, :],
                                                in1=bk, op=OP.add)

            def emit_gather(i, st):
                x = dbl3.tile([P, S, D], BF16, tag="x")
                nc.sync.dma_start(
                    x[:].rearrange("p s d -> p (s d)"),
                    xg_d[ds(i * P, P), :])
                st[i] = {'x': x}

            def emit_A_mm(i, l, st):
                """transposes + QKV matmuls for (i, l)."""
                x = st[i]['x']
                xT = [dbl3.tile([P, DC, 4 * P], BF16, tag="xT",
                                name=f"xT{h}") for h in range(2)]
                if l == 0:
                    for h in range(2):
                        nc.sync.dma_start(
                            xT[h][:],
                            xt0_d[i, :, :, h * 4 * P:(h + 1) * 4 * P]
                            .transpose([1, 0, 2]))
                else:
                    for s in range(S):
                        for c in range(DC):
                            nc.sync.dma_start_transpose(
                                xT[s // 4][:, c, (s % 4) * P:(s % 4 + 1) * P],
                                x[:, s, c * P:(c + 1) * P])
                pqs = []
                for s in range(S):
                    pq = psA.tile([P, 3 * D], F32, tag="pqkv")
                    for c in range(DC):
                        lhsT = xT[s // 4][:, c, (s % 4) * P:(s % 4 + 1) * P]
                        for nb in range(3):
                            nc.tensor.matmul(
                                pq[:, nb * D:(nb + 1) * D], lhsT,
                                wq_sb[l][:, c, nb * D:(nb + 1) * D],
                                start=(c == 0), stop=(c == DC - 1))
                    pqs.append(pq)
                st[i][f'pq{l}'] = pqs

            def emit_A_cp(i, l, st):
                # qkv laid [P, 3, S, D]: q/k/v slabs each contiguous
                qkv = dbl.tile([P, 3, S, D], BF16, tag="qkv")
                for s, pq in enumerate(st[i][f'pq{l}']):
                    if flags['bqkv']:
                        nc.vector.tensor_add(pq[:], pq[:],
                                             vsl('bqkv', l, 3 * D))
                    nc.scalar.copy(
                        qkv[:, :, s, :],
                        pq[:].rearrange("p (n d) -> p n d", n=3))
                st[i]['qkv'] = qkv

            def emit_A(i, l, st):
                emit_A_mm(i, l, st)
                emit_A_cp(i, l, st)

            def emit_Bq(i, l, st):
                """attention scores + softmax + pn2 for (i, l)."""
                qkv = st[i]['qkv']
                scores = wp.tile([P, S, H, S], BF16, tag="scores")
                for s in range(S):
                    qk = dbl.tile([P, S, H, DH], BF16, tag="qks")
                    nc.vector.tensor_tensor(
                        out=qk[:],
                        in0=qkv[:, 1, :, :]
                            .rearrange("p t (h e) -> p t h e", h=H),
                        in1=qkv[:, 0, s, :]
                            .rearrange("p (h e) -> p h e", h=H)
                            .unsqueeze(1).broadcast_to([P, S, H, DH]),
                        op=OP.mult)
                    for w in (32, 16, 8):
                        nc.vector.tensor_tensor(out=qk[:, :, :, 0:w],
                                                in0=qk[:, :, :, 0:w],
                                                in1=qk[:, :, :, w:2 * w],
                                                op=OP.add)
                    with nc.allow_low_precision("bf16 scores ok at 2e-2"):
                        nc.vector.reduce_sum(
                            scores[:, s, :, :].transpose([0, 2, 1]),
                            qk[:, :, :, 0:8], axis=AX.X)

                # softmax over t (scores O(1): skip max-sub); exp in place
                pnf = wp.tile([P, S * H], F32, tag="den")
                nc.scalar.activation(
                    scores[:].rearrange("p s h t -> p (s h t)"),
                    scores[:].rearrange("p s h t -> p (s h t)"), AF.Exp)
                den = pnf[:].rearrange("p (s h) -> p s h", s=S)
                nc.vector.reduce_sum(den, scores[:], axis=AX.X)
                nc.vector.reciprocal(pnf[:], pnf[:])
                # pn laid [P, s, t, h] (strided write, (s,h,t) iteration)
                pn = wp.tile([P, S, S, H], BF16, tag="pn")
                nc.vector.tensor_tensor(
                    out=pn[:].transpose([0, 1, 3, 2]), in0=scores[:],
                    in1=den.unsqueeze(3).broadcast_to([P, S, H, S]),
                    op=OP.mult)
                # x2-packed replica [P, s, t, h, 2], written per-s (3D APs)
                pn2 = sp.tile([P, S, S, H, 2], BF16, tag="pn2")
                for s in range(S):
                    if PN2_ACT:
                        nc.scalar.copy(
                            pn2[:, s, :, :, :],
                            pn[:, s, :, :].unsqueeze(3)
                            .broadcast_to([P, S, H, 2]))
                    else:
                        nc.vector.tensor_copy(
                            pn2[:, s, :, :, :],
                            pn[:, s, :, :].unsqueeze(3)
                            .broadcast_to([P, S, H, 2]))
                st[i]['pn2'] = pn2

            def emit_Bv(i, l, st):
                """AV + Wo + LN1 + FFN1 + FFN2-matmuls for (i, l)."""
                x = st[i]['x']
                qkv = st[i]['qkv']
                pn2 = st[i]['pn2']
                aT = [dbl.tile([P, DC, 4 * P], BF16, tag="aT2",
                               name=f"aTh{h}") for h in range(2)]
                sums1 = wp.tile([P, S], F32, tag="sums1")
                pos = [None] * S

                def resid1(s):
                    nc.vector.scalar_tensor_tensor(
                        out=x[:, s, :], in0=pos[s][:], scalar=1.0,
                        in1=x[:, s, :], op0=OP.mult, op1=OP.add,
                        accum_out=sums1[:, s:s + 1])

                for s in range(S):
                    av = dbl.tile([P, S, H, DH], BF16, tag="qks")
                    # 3-free-dim APs: (t h) merged, e split as 32x2
                    pn_in = pn2[:, s, :, :, :] \
                        .rearrange("p t h b -> p (t h) b") \
                        .unsqueeze(2).broadcast_to([P, S * H, 32, 2])
                    nc.vector.tensor_tensor(
                        out=av[:].rearrange("p t h (a b) -> p (t h) a b",
                                            b=2),
                        in0=qkv[:, 2, :, :]
                            .rearrange("p t (h a b) -> p (t h) a b",
                                       h=H, b=2),
                        in1=pn_in,
                        op=OP.mult)
                    # t-reduction on PE: identity-matmul PSUM accumulation
                    pa = psB.tile([P, D], F32, tag="mm")
                    for t in range(S):
                        nc.tensor.matmul(
                            pa[:], id_sb[:],
                            av[:, t, :, :].rearrange("p h e -> p (h e)"),
                            start=(t == 0), stop=(t == S - 1))
                    asb = wp.tile([P, D], BF16, tag="asb")
                    nc.scalar.copy(asb[:], pa[:])
                    for c in range(DC):
                        nc.sync.dma_start_transpose(
                            aT[s // 4][:, c, (s % 4) * P:(s % 4 + 1) * P],
                            asb[:, c * P:(c + 1) * P])
                    po = psB.tile([P, D], F32, tag="mm")
                    for c in range(DC):
                        nc.tensor.matmul(
                            po[:],
                            aT[s // 4][:, c, (s % 4) * P:(s % 4 + 1) * P],
                            wo_sb[l][:, c, :],
                            start=(c == 0), stop=(c == DC - 1))
                    if flags['bo']:
                        nc.vector.tensor_add(po[:], po[:], vsl('bo', l, D))
                    pos[s] = po
                    if s >= 2:
                        resid1(s - 2)
                resid1(S - 2)
                resid1(S - 1)

                layernorm(x, sums1, l,
                          vsl('ln1_g', l, D) if flags['ln_g'] else None,
                          vsl('ln1_b', l, D) if flags['ln_b'] else None)

                # x1^T for FFN1
                x1T = [dbl.tile([P, DC, 4 * P], BF16, tag="aT2",
                                name=f"x1Th{h}") for h in range(2)]
                for s in range(S):
                    for c in range(DC):
                        nc.sync.dma_start_transpose(
                            x1T[s // 4][:, c, (s % 4) * P:(s % 4 + 1) * P],
                            x[:, s, c * P:(c + 1) * P])

                # FFN1 -> h^T [dff-part, tok], fused relu (+b1)
                hT = wp.tile([P, FCH, S * P], BF16, tag="hT")
                for m in range(FCH):
                    for hf in range(2):
                        ph = psB.tile([P, D], F32, tag="mm")
                        for c in range(DC):
                            nc.tensor.matmul(
                                ph[:],
                                w1_sb[l][:, c, m * P:(m + 1) * P],
                                x1T[hf][:, c, :],
                                start=(c == 0), stop=(c == DC - 1))
                        if flags['b1']:
                            nc.scalar.activation(
                                hT[:, m, hf * D:(hf + 1) * D], ph[:],
                                AF.Relu,
                                bias=b1t_sb[:, l * FCH + m:l * FCH + m + 1])
                        else:
                            nc.scalar.activation(
                                hT[:, m, hf * D:(hf + 1) * D], ph[:],
                                AF.Relu)

                # FFN2 matmuls; results staged to SBUF (fb) by ACT so the
                # DVE-side residual can run after the next attention block
                fb = wp.tile([P, S, D], BF16, tag="fb")
                for s in range(S):
                    pf = psB.tile([P, D], F32, tag="mm")
                    for k in range(FCH):
                        nc.tensor.matmul(pf[:],
                                         hT[:, k, s * P:(s + 1) * P],
                                         w2_sb[l][:, k, :],
                                         start=(k == 0),
                                         stop=(k == FCH - 1))
                    if flags['b2']:
                        nc.vector.tensor_add(pf[:], pf[:], vsl('b2', l, D))
                    nc.scalar.copy(fb[:, s, :], pf[:])
                st[i][f'fb{l}'] = fb

            def emit_C(i, l, st):
                """FFN2 residual (from staged fb) + LN2."""
                x = st[i]['x']
                fb = st[i][f'fb{l}']
                sums2 = wp.tile([P, S], F32, tag="sums2")
                for s in range(S):
                    nc.vector.scalar_tensor_tensor(
                        out=x[:, s, :], in0=fb[:, s, :], scalar=1.0,
                        in1=x[:, s, :], op0=OP.mult, op1=OP.add,
                        accum_out=sums2[:, s:s + 1])
                layernorm(x, sums2, l,
                          vsl('ln2_g', l, D) if flags['ln_g'] else None,
                          vsl('ln2_b', l, D) if flags['ln_b'] else None)

            def emit_epi(i, st):
                x = st[i]['x']
                nc.vector.tensor_tensor(out=x[:, 0:4, :], in0=x[:, 0:4, :],
                                        in1=x[:, 4:8, :], op=OP.max)
                nc.vector.tensor_tensor(out=x[:, 0:2, :], in0=x[:, 0:2, :],
                                        in1=x[:, 2:4, :], op=OP.max)
                rst = wp.tile([P, D], BF16, tag="rst")
                nc.vector.tensor_tensor(out=rst[:], in0=x[:, 0, :],
                                        in1=x[:, 1, :], op=OP.max)
                rT = wp.tile([P, DC, P], BF16, tag="asb")
                for c in range(DC):
                    nc.sync.dma_start_transpose(rT[:, c, :],
                                                rst[:, c * P:(c + 1) * P])
                pc = psB.tile([P, D], F32, tag="mm")
                for c in range(DC):
                    nc.tensor.matmul(pc[:, 0:NCLS], rT[:, c, :], wf_sb[:, c, :],
                                     start=(c == 0), stop=(c == DC - 1))
                if flags['bfc']:
                    nc.vector.tensor_add(pc[:, 0:NCLS], pc[:, 0:NCLS],
                                         vec_sb['bfc'][:, :])
                lg = wp.tile([P, NCLS], F32, tag="lg")
                nc.scalar.copy(lg[:], pc[:, 0:NCLS])
                nc.sync.dma_start(out_d[ds(i * P, P), :], lg[:])

            # ---- software-pipelined emission ----
            st = {}
            emit_gather(0, st)
            emit_A(0, 0, st)
            if NT > 1:
                emit_gather(1, st)
                emit_A(1, 0, st)
            emit_Bq(0, 0, st)
            emit_Bv(0, 0, st)
            emit_C(0, 0, st)
            for i in range(NT):
                emit_A(i, 1, st)
                if i + 2 < NT:
                    emit_gather(i + 2, st)
                    emit_A_mm(i + 2, 0, st)
                if i + 1 < NT:
                    emit_Bq(i + 1, 0, st)
                    emit_Bv(i + 1, 0, st)
                emit_Bq(i, 1, st)
                emit_Bv(i, 1, st)
                if i + 2 < NT:
                    emit_A_cp(i + 2, 0, st)
                if i + 1 < NT:
                    emit_C(i + 1, 0, st)
                emit_C(i, 1, st)
                emit_epi(i, st)

    _split_multiwait_drains(nc)
    return nc


OPT_KEYS = ('bqkv', 'bo', 'b1', 'b2', 'bfc', 'ln_g', 'ln_b')
_cache = {}


def _get_nc(flags):
    key = tuple(flags[k] for k in OPT_KEYS)
    if key not in _cache:
        _cache[key] = build(flags)
    return _cache[key]


def _prep_inputs(inputs, flags):
    bf = ml_dtypes.bfloat16
    token_ids = np.asarray(inputs['token_ids'])
    edge_src = np.asarray(inputs['edge_src'])
    emb = np.asarray(inputs['emb'], dtype=np.float32)
    Wqkv = np.asarray(inputs['Wqkv'], dtype=np.float32)
    Wo = np.asarray(inputs['Wo'], dtype=np.float32)
    W1 = np.asarray(inputs['W1'], dtype=np.float32)
    W2 = np.asarray(inputs['W2'], dtype=np.float32)
    Wfc = np.asarray(inputs['Wfc'], dtype=np.float32)

    tid2 = token_ids[edge_src[:, :S]]                      # [NDST, S]
    xg = emb.astype(bf)[tid2]                              # [NDST, S, D]
    wqkvT = np.ascontiguousarray(Wqkv.transpose(0, 2, 1))  # [NL, D, 3D]
    wqkvT[:, :, 0:D] *= 0.125                              # fold q scale
    common = {
        'wqkvT': wqkvT.astype(bf),
        'woT': np.ascontiguousarray(Wo.transpose(0, 2, 1)).astype(bf),
        'w1T': np.ascontiguousarray(W1.transpose(0, 2, 1)).astype(bf),
        'w2T': np.ascontiguousarray(W2.transpose(0, 2, 1)).astype(bf),
        'wfcT': np.ascontiguousarray(Wfc.T).astype(bf),
        'ident': np.eye(P, dtype=bf),
    }
    if flags['bqkv']:
        bq = np.asarray(inputs['bqkv'], dtype=np.float32).copy()
        bq[:, 0:D] *= 0.125
        common['bqkv'] = np.ascontiguousarray(
            np.broadcast_to(bq.reshape(1, -1), (P, NL * 3 * D)))
    if flags['bo']:
        common['bo'] = np.ascontiguousarray(np.broadcast_to(
            np.asarray(inputs['bo'], np.float32).reshape(1, -1),
            (P, NL * D)))
    if flags['b1']:
        b1 = np.asarray(inputs['b1'], dtype=np.float32)
        common['b1t'] = np.ascontiguousarray(
            b1.reshape(NL, FCH, P).transpose(2, 0, 1).reshape(P, NL * FCH))
    if flags['b2']:
        common['b2'] = np.ascontiguousarray(np.broadcast_to(
            np.asarray(inputs['b2'], np.float32).reshape(1, -1),
            (P, NL * D)))
    if flags['bfc']:
        common['bfc'] = np.ascontiguousarray(np.broadcast_to(
            np.asarray(inputs['bfc'], np.float32).reshape(1, -1),
            (P, NCLS)))
    if flags['ln_g']:
        common['ln1_g'] = np.ascontiguousarray(np.broadcast_to(
            np.asarray(inputs['ln1_g'], np.float32).reshape(1, -1),
            (P, NL * D)))
        common['ln2_g'] = np.ascontiguousarray(np.broadcast_to(
            np.asarray(inputs['ln2_g'], np.float32).reshape(1, -1),
            (P, NL * D)))
    if flags['ln_b']:
        common['ln1_b'] = np.ascontiguousarray(np.broadcast_to(
            np.asarray(inputs['ln1_b'], np.float32).reshape(1, -1),
            (P, NL * D)))
        common['ln2_b'] = np.ascontiguousarray(np.broadcast_to(
            np.asarray(inputs['ln2_b'], np.float32).reshape(1, -1),
            (P, NL * D)))

    in_maps = []
    for c in range(NCORES):
        m = dict(common)
        xc = xg[c * NLOC:(c + 1) * NLOC]                   # [NLOC, S, D]
        m['xg'] = np.ascontiguousarray(xc.reshape(NLOC, S * D))
        # xt0[i, c, p, s*128+n] = x[i*128+n, s, c*128+p]
        m['xt0'] = np.ascontiguousarray(
            xc.reshape(NT, P, S, DC, P).transpose(0, 3, 4, 2, 1)
            .reshape(NT, DC, P, S * P))
        in_maps.append(m)
    return in_maps


def _get_flags(inputs):
    return {
        'bqkv': bool(np.any(inputs['bqkv'])),
        'bo': bool(np.any(inputs['bo'])),
        'b1': bool(np.any(inputs['b1'])),
        'b2': bool(np.any(inputs['b2'])),
        'bfc': bool(np.any(inputs['bfc'])),
        'ln_g': bool(np.any(np.asarray(inputs['ln1_g']) != 1.0)
                     or np.any(np.asarray(inputs['ln2_g']) != 1.0)),
        'ln_b': bool(np.any(inputs['ln1_b']) or np.any(inputs['ln2_b'])),
    }


def kernel(**inputs):
    flags = _get_flags(inputs)
    nc = _get_nc(flags)
    in_maps = _prep_inputs(inputs, flags)
    res = run_bass_kernel_spmd(nc, in_maps, core_ids=list(range(NCORES)))
    out = np.concatenate([res.results[c]['logits'] for c in range(NCORES)],
                         axis=0)
    return out.astype(np.float32)


if __name__ == '__main__':
    import time
    sys.path.insert(0, '/root/problem')
    import reference
    inp = {k: np.asarray(v) for k, v in reference.setup_inputs().items()}
    t0 = time.time()
    got = kernel(**inp)
    print(f"kernel ran in {time.time()-t0:.1f}s")
    exp = np.asarray(reference.reference(**reference.setup_inputs()))
    err = np.abs(got - exp).max()
    rel = err / np.abs(exp).max()
    print(f"absmax err {err:.3e}  rel {rel:.3e}")


# revision 8
# speedup vs baseline: 4.5080x; 1.0279x over previous
"""GTE program-classification kernel v2 for 8 Trainium2 NeuronCores.

Data-parallel over dst nodes: each core handles 1024 of the 8192 dst nodes.

Key design points (vs the v1 baseline):
  - mailbox gather + the layer-0 activation transpose are precomputed on the
    host (pure indexing / layout), so the device kernel starts from plain
    contiguous DMA loads -- no indirect-DMA machinery.
  - q is pre-scaled into Wq; one fused PSUM->SBUF qkv copy per s on ACT
    (qkv laid [P, 3, S, D] so q/k/v slabs are each contiguous).
  - attention scores: per-s DVE multiply ladder in 2x bf16 mode, tail via
    TensorReduce; softmax without max-subtraction (scores are O(1)).
  - AV multiply keeps DVE 2x mode via a x2-packed pn replica (pn2) with
    3-free-dim APs; the t-reduction runs on the PE as an identity-matmul
    PSUM accumulation (8 x 512-col matmuls per s) instead of DVE adds.
  - residual adds fused with the PSUM eviction via scalar_tensor_tensor,
    whose accum_out yields the LN mean for free; LN sum-of-squares on ACT
    (Square+accum), normalize via 4x TensorScalarPtr.
  - 2-stage software pipeline across node tiles: tile i+1's layer-0 unit
    runs interleaved with tile i's layer-1 unit; QKV of the next units is
    prefetched into the attention windows; the FFN2 residual is deferred
    past the next attention block via an SBUF staging tile (fb).
GPSIMD is not used for compute (walrus rejects generic ops on Pool).
"""
import sys
if '/opt/trn_rl_repo' not in sys.path:
    sys.path.insert(0, '/opt/trn_rl_repo')

import numpy as np
import ml_dtypes

import concourse.bass as bass
import concourse.tile as tile
import concourse.mybir as mybir
from concourse.bass import ds
from concourse.bass_utils import run_bass_kernel_spmd

F32 = mybir.dt.float32
BF16 = mybir.dt.bfloat16
I32 = mybir.dt.int32
AF = mybir.ActivationFunctionType
OP = mybir.AluOpType
AX = mybir.AxisListType

P = 128
D = 512
H = 8
DH = 64
S = 8          # messages used per node (9th dropped by the reference)
NL = 2
V = 50000
NCLS = 104
DFF = 1024
NDST = 8192
NSRC = 40000
NCORES = 8
NLOC = NDST // NCORES      # 1024 dst nodes per core
NT = NLOC // P             # 8 node tiles per core
DC = D // P                # 4 d-chunks
FCH = DFF // P             # 8 dff-chunks
LN_EPS = 1e-5

NORM_ACT = False    # LN normalize on ACT instead of DVE TensorScalarPtr
PN2_ACT = True      # pn2 replica on ACT (else DVE)


def _split_multiwait_drains(nc):
    """walrus in this container accepts only one sync-wait per instruction;
    split any multi-wait Drain into a chain of single-wait drains."""
    for fn in nc.m.functions:
        for bb in fn.blocks:
            newlist = []
            for ins in bb.instructions:
                si = ins.sync_info
                if si is not None and si.on_wait and len(si.on_wait) > 1:
                    waits = list(si.on_wait)
                    for j, w in enumerate(waits[:-1]):
                        d = mybir.InstDrain(name=f'{ins.name}-sw{j}',
                                            engine=ins.engine)
                        d.sync_info = mybir.SyncInfo(on_wait=[w], on_update=[])
                        newlist.append(d)
                    si.on_wait = [waits[-1]]
                newlist.append(ins)
            bb.instructions[:] = newlist


def build(flags):
    nc = bass.Bass()

    xg_d = nc.dram_tensor("xg", [NLOC, S * D], BF16, kind="ExternalInput")
    xt0_d = nc.dram_tensor("xt0", [NT, DC, P, S * P], BF16,
                           kind="ExternalInput")
    wq_d = nc.dram_tensor("wqkvT", [NL, D, 3 * D], BF16, kind="ExternalInput")
    wo_d = nc.dram_tensor("woT", [NL, D, D], BF16, kind="ExternalInput")
    w1_d = nc.dram_tensor("w1T", [NL, D, DFF], BF16, kind="ExternalInput")
    w2_d = nc.dram_tensor("w2T", [NL, DFF, D], BF16, kind="ExternalInput")
    wf_d = nc.dram_tensor("wfcT", [D, NCLS], BF16, kind="ExternalInput")
    id_d = nc.dram_tensor("ident", [P, P], BF16, kind="ExternalInput")
    out_d = nc.dram_tensor("logits", [NLOC, NCLS], F32, kind="ExternalOutput")

    need_vec = {}
    if flags['bqkv']:
        need_vec['bqkv'] = [NL, 3 * D]
    if flags['bo']:
        need_vec['bo'] = [NL, D]
    if flags['b2']:
        need_vec['b2'] = [NL, D]
    if flags['bfc']:
        need_vec['bfc'] = [1, NCLS]
    if flags['ln_g']:
        need_vec['ln1_g'] = [NL, D]
        need_vec['ln2_g'] = [NL, D]
    if flags['ln_b']:
        need_vec['ln1_b'] = [NL, D]
        need_vec['ln2_b'] = [NL, D]
    # replicated vectors: host passes them pre-broadcast as [P, n]
    vec_d = {k: nc.dram_tensor(k, [P, shp[0] * shp[1]], F32,
                               kind="ExternalInput")
             for k, shp in need_vec.items()}
    b1t_d = (nc.dram_tensor("b1t", [P, NL * FCH], F32, kind="ExternalInput")
             if flags['b1'] else None)

    with tile.TileContext(nc) as tc:
        with tc.tile_pool(name="wpool", bufs=1) as wp, \
             tc.tile_pool(name="dbl", bufs=2) as dbl, \
             tc.tile_pool(name="dbl3", bufs=3) as dbl3, \
             tc.tile_pool(name="sp", bufs=2) as sp, \
             tc.tile_pool(name="psA", bufs=2, space="PSUM") as psA, \
             tc.tile_pool(name="psB", bufs=2, space="PSUM") as psB:

            # ---- resident weights (bf16) ----
            wq_sb, wo_sb, w1_sb, w2_sb = [], [], [], []
            for l in range(NL):
                t = wp.tile([P, DC, 3 * D], BF16, tag=f"wq{l}")
                for c in range(DC):
                    nc.sync.dma_start(t[:, c, :], wq_d[l, c * P:(c + 1) * P, :])
                wq_sb.append(t)
                t = wp.tile([P, DC, D], BF16, tag=f"wo{l}")
                for c in range(DC):
                    nc.sync.dma_start(t[:, c, :], wo_d[l, c * P:(c + 1) * P, :])
                wo_sb.append(t)
                t = wp.tile([P, DC, DFF], BF16, tag=f"w1{l}")
                for c in range(DC):
                    nc.sync.dma_start(t[:, c, :], w1_d[l, c * P:(c + 1) * P, :])
                w1_sb.append(t)
                t = wp.tile([P, FCH, D], BF16, tag=f"w2{l}")
                for c in range(FCH):
                    nc.sync.dma_start(t[:, c, :], w2_d[l, c * P:(c + 1) * P, :])
                w2_sb.append(t)
            wf_sb = wp.tile([P, DC, NCLS], BF16, tag="wf")
            for c in range(DC):
                nc.sync.dma_start(wf_sb[:, c, :], wf_d[c * P:(c + 1) * P, :])
            id_sb = wp.tile([P, P], BF16, tag="ident")
            nc.sync.dma_start(id_sb[:], id_d[:])

            vec_sb = {}
            for k in need_vec:
                tb = wp.tile([P, vec_d[k].shape[1]], F32, tag=f"{k}_rep")
                nc.sync.dma_start(tb[:], vec_d[k][:])
                vec_sb[k] = tb

            b1t_sb = None
            if flags['b1']:
                b1t_sb = wp.tile([P, NL * FCH], F32, tag="b1t")
                nc.sync.dma_start(b1t_sb[:], b1t_d[:])

            def vsl(k, l, n):
                return vec_sb[k][:, l * n:(l + 1) * n]

            def layernorm(x, sums, l, gk, bk):
                """x: [P,S,D] bf16 post-residual; sums: [P,S] f32 row-sums.
                In-place normalize; sum-of-squares on ACT."""
                sq = wp.tile([P, D], BF16, tag="lnsq")
                st = wp.tile([P, 4 * S], F32, tag="lnstat")
                qs = st[:, 0:S]
                nmean = st[:, S:2 * S]
                var = st[:, 2 * S:3 * S]
                rstd = st[:, 3 * S:4 * S]
                for s in range(S):
                    nc.scalar.activation(sq[:], x[:, s, :], AF.Square,
                                         accum_out=qs[:, s:s + 1])
                nc.vector.tensor_scalar_mul(nmean[:], sums[:], -1.0 / D)
                nc.vector.tensor_tensor(out=var[:], in0=nmean[:], in1=nmean[:],
                                        op=OP.mult)
                nc.vector.scalar_tensor_tensor(
                    out=var[:], in0=qs[:], scalar=1.0 / D, in1=var[:],
                    op0=OP.mult, op1=OP.subtract)
                nc.vector.tensor_scalar_add(var[:], var[:], LN_EPS)
                nc.vector.reciprocal(var[:], var[:])
                nc.scalar.activation(rstd[:], var[:], AF.Sqrt)
                for s in range(S):
                    nc.vector.tensor_scalar(
                        out=x[:, s, :], in0=x[:, s, :],
                        scalar1=nmean[:, s:s + 1],
                        scalar2=rstd[:, s:s + 1],
                        op0=OP.add, op1=OP.mult)
                if gk is not None:
                    for s in range(S):
                        nc.vector.tensor_tensor(out=x[:, s, :], in0=x[:, s, :],
                                                in1=gk, op=OP.mult)
                if bk is not None:
                    for s in range(S):
                        nc.vector.tensor_tensor(out=x[:, s, :], in0=x[:, s, :],
                                                in1=bk, op=OP.add)

            def emit_gather(i, st):
                x = dbl3.tile([P, S, D], BF16, tag="x")
                nc.sync.dma_start(
                    x[:].rearrange("p s d -> p (s d)"),
                    xg_d[ds(i * P, P), :])
                st[i] = {'x': x}

            def emit_A_mm(i, l, st):
                """transposes + QKV matmuls for (i, l)."""
                x = st[i]['x']
                xT = [dbl3.tile([P, DC, 4 * P], BF16, tag="xT",
                                name=f"xT{h}") for h in range(2)]
                if l == 0:
                    for h in range(2):
                        nc.sync.dma_start(
                            xT[h][:],
                            xt0_d[i, :, :, h * 4 * P:(h + 1) * 4 * P]
                            .transpose([1, 0, 2]))
                else:
                    for s in range(S):
                        for c in range(DC):
                            nc.sync.dma_start_transpose(
                                xT[s // 4][:, c, (s % 4) * P:(s % 4 + 1) * P],
                                x[:, s, c * P:(c + 1) * P])
                pqs = []
                for s in range(S):
                    pq = psA.tile([P, 3 * D], F32, tag="pqkv")
                    for c in range(DC):
                        lhsT = xT[s // 4][:, c, (s % 4) * P:(s % 4 + 1) * P]
                        for nb in range(3):
                            nc.tensor.matmul(
                                pq[:, nb * D:(nb + 1) * D], lhsT,
                                wq_sb[l][:, c, nb * D:(nb + 1) * D],
                                start=(c == 0), stop=(c == DC - 1))
                    pqs.append(pq)
                st[i][f'pq{l}'] = pqs

            def emit_A_cp(i, l, st):
                # qkv laid [P, 3, S, D]: q/k/v slabs each contiguous
                qkv = dbl.tile([P, 3, S, D], BF16, tag="qkv")
                for s, pq in enumerate(st[i][f'pq{l}']):
                    if flags['bqkv']:
                        nc.vector.tensor_add(pq[:], pq[:],
                                             vsl('bqkv', l, 3 * D))
                    nc.scalar.copy(
                        qkv[:, :, s, :],
                        pq[:].rearrange("p (n d) -> p n d", n=3))
                st[i]['qkv'] = qkv

            def emit_A(i, l, st):
                emit_A_mm(i, l, st)
                emit_A_cp(i, l, st)

            def emit_Bq(i, l, st):
                """attention scores + softmax + pn2 for (i, l)."""
                qkv = st[i]['qkv']
                scores = wp.tile([P, S, H, S], BF16, tag="scores")
                for s in range(S):
                    for th in range(2):
                        qk = dbl.tile([P, 4, H, DH], BF16, tag="qkh",
                                      name=f"qkh{th}")
                        nc.vector.tensor_tensor(
                            out=qk[:],
                            in0=qkv[:, 1, th * 4:(th + 1) * 4, :]
                                .rearrange("p t (h e) -> p t h e", h=H),
                            in1=qkv[:, 0, s, :]
                                .rearrange("p (h e) -> p h e", h=H)
                                .unsqueeze(1).broadcast_to([P, 4, H, DH]),
                            op=OP.mult)
                        for w in (32, 16, 8):
                            nc.vector.tensor_tensor(out=qk[:, :, :, 0:w],
                                                    in0=qk[:, :, :, 0:w],
                                                    in1=qk[:, :, :, w:2 * w],
                                                    op=OP.add)
                        with nc.allow_low_precision("bf16 scores ok"):
                            nc.vector.reduce_sum(
                                scores[:, s, :, th * 4:(th + 1) * 4]
                                .transpose([0, 2, 1]),
                                qk[:, :, :, 0:8], axis=AX.X)

                # softmax over t (scores O(1): skip max-sub); exp in place
                pnf = wp.tile([P, S * H], F32, tag="den")
                nc.scalar.activation(
                    scores[:].rearrange("p s h t -> p (s h t)"),
                    scores[:].rearrange("p s h t -> p (s h t)"), AF.Exp)
                den = pnf[:].rearrange("p (s h) -> p s h", s=S)
                nc.vector.reduce_sum(den, scores[:], axis=AX.X)
                nc.vector.reciprocal(pnf[:], pnf[:])
                # pn laid [P, s, t, h] (strided write, (s,h,t) iteration)
                pn = wp.tile([P, S, S, H], BF16, tag="pn")
                nc.vector.tensor_tensor(
                    out=pn[:].transpose([0, 1, 3, 2]), in0=scores[:],
                    in1=den.unsqueeze(3).broadcast_to([P, S, H, S]),
                    op=OP.mult)
                # x2-packed replica [P, s, t, h, 2], written per-s (3D APs)
                pn2 = sp.tile([P, S, S, H, 2], BF16, tag="pn2")
                for s in range(S):
                    if PN2_ACT:
                        nc.scalar.copy(
                            pn2[:, s, :, :, :],
                            pn[:, s, :, :].unsqueeze(3)
                            .broadcast_to([P, S, H, 2]))
                    else:
                        nc.vector.tensor_copy(
                            pn2[:, s, :, :, :],
                            pn[:, s, :, :].unsqueeze(3)
                            .broadcast_to([P, S, H, 2]))
                st[i]['pn2'] = pn2

            def emit_Bv(i, l, st):
                """AV + Wo + LN1 + FFN1 + FFN2-matmuls for (i, l)."""
                x = st[i]['x']
                qkv = st[i]['qkv']
                pn2 = st[i]['pn2']
                aT = [dbl.tile([P, DC, 4 * P], BF16, tag="aT2",
                               name=f"aTh{h}") for h in range(2)]
                sums1 = wp.tile([P, S], F32, tag="sums1")
                pos = [None] * S

                def resid1(s):
                    nc.vector.scalar_tensor_tensor(
                        out=x[:, s, :], in0=pos[s][:], scalar=1.0,
                        in1=x[:, s, :], op0=OP.mult, op1=OP.add,
                        accum_out=sums1[:, s:s + 1])

                for s in range(S):
                    avh = []
                    for th in range(2):
                        av = dbl.tile([P, 4, H, DH], BF16, tag="avh",
                                      name=f"avh{th}")
                        # 3-free-dim APs: (t h) merged, e split as 32x2
                        pn_in = pn2[:, s, th * 4:(th + 1) * 4, :, :] \
                            .rearrange("p t h b -> p (t h) b") \
                            .unsqueeze(2).broadcast_to([P, 4 * H, 32, 2])
                        nc.vector.tensor_tensor(
                            out=av[:].rearrange(
                                "p t h (a b) -> p (t h) a b", b=2),
                            in0=qkv[:, 2, th * 4:(th + 1) * 4, :]
                                .rearrange("p t (h a b) -> p (t h) a b",
                                           h=H, b=2),
                            in1=pn_in,
                            op=OP.mult)
                        avh.append(av)
                    # t-reduction on PE: identity-matmul PSUM accumulation
                    pa = psB.tile([P, D], F32, tag="mm")
                    for t in range(S):
                        nc.tensor.matmul(
                            pa[:], id_sb[:],
                            avh[t // 4][:, t % 4, :, :]
                            .rearrange("p h e -> p (h e)"),
                            start=(t == 0), stop=(t == S - 1))
                    asb = wp.tile([P, D], BF16, tag="asb")
                    nc.scalar.copy(asb[:], pa[:])
                    for c in range(DC):
                        nc.sync.dma_start_transpose(
                            aT[s // 4][:, c, (s % 4) * P:(s % 4 + 1) * P],
                            asb[:, c * P:(c + 1) * P])
                    po = psB.tile([P, D], F32, tag="mm")
                    for c in range(DC):
                        nc.tensor.matmul(
                            po[:],
                            aT[s // 4][:, c, (s % 4) * P:(s % 4 + 1) * P],
                            wo_sb[l][:, c, :],
                            start=(c == 0), stop=(c == DC - 1))
                    if flags['bo']:
                        nc.vector.tensor_add(po[:], po[:], vsl('bo', l, D))
                    pos[s] = po
                    if s >= 2:
                        resid1(s - 2)
                resid1(S - 2)
                resid1(S - 1)

                layernorm(x, sums1, l,
                          vsl('ln1_g', l, D) if flags['ln_g'] else None,
                          vsl('ln1_b', l, D) if flags['ln_b'] else None)

                # x1^T for FFN1
                x1T = [dbl.tile([P, DC, 4 * P], BF16, tag="aT2",
                                name=f"x1Th{h}") for h in range(2)]
                for s in range(S):
                    for c in range(DC):
                        nc.sync.dma_start_transpose(
                            x1T[s // 4][:, c, (s % 4) * P:(s % 4 + 1) * P],
                            x[:, s, c * P:(c + 1) * P])

                # FFN1 -> h^T [dff-part, tok], fused relu (+b1)
                hT = wp.tile([P, FCH, S * P], BF16, tag="hT")
                for m in range(FCH):
                    for hf in range(2):
                        ph = psB.tile([P, D], F32, tag="mm")
                        for c in range(DC):
                            nc.tensor.matmul(
                                ph[:],
                                w1_sb[l][:, c, m * P:(m + 1) * P],
                                x1T[hf][:, c, :],
                                start=(c == 0), stop=(c == DC - 1))
                        if flags['b1']:
                            nc.scalar.activation(
                                hT[:, m, hf * D:(hf + 1) * D], ph[:],
                                AF.Relu,
                                bias=b1t_sb[:, l * FCH + m:l * FCH + m + 1])
                        else:
                            nc.scalar.activation(
                                hT[:, m, hf * D:(hf + 1) * D], ph[:],
                                AF.Relu)

                # FFN2 matmuls; results staged to SBUF (fb) by ACT so the
                # DVE-side residual can run after the next attention block
                fb = wp.tile([P, S, D], BF16, tag="fb")
                for s in range(S):
                    pf = psB.tile([P, D], F32, tag="mm")
                    for k in range(FCH):
                        nc.tensor.matmul(pf[:],
                                         hT[:, k, s * P:(s + 1) * P],
                                         w2_sb[l][:, k, :],
                                         start=(k == 0),
                                         stop=(k == FCH - 1))
                    if flags['b2']:
                        nc.vector.tensor_add(pf[:], pf[:], vsl('b2', l, D))
                    nc.scalar.copy(fb[:, s, :], pf[:])
                st[i][f'fb{l}'] = fb

            def emit_C(i, l, st):
                """FFN2 residual (from staged fb) + LN2."""
                x = st[i]['x']
                fb = st[i][f'fb{l}']
                sums2 = wp.tile([P, S], F32, tag="sums2")
                for s in range(S):
                    nc.vector.scalar_tensor_tensor(
                        out=x[:, s, :], in0=fb[:, s, :], scalar=1.0,
                        in1=x[:, s, :], op0=OP.mult, op1=OP.add,
                        accum_out=sums2[:, s:s + 1])
                layernorm(x, sums2, l,
                          vsl('ln2_g', l, D) if flags['ln_g'] else None,
                          vsl('ln2_b', l, D) if flags['ln_b'] else None)

            def emit_epi(i, st):
                x = st[i]['x']
                nc.vector.tensor_tensor(out=x[:, 0:4, :], in0=x[:, 0:4, :],
                                        in1=x[:, 4:8, :], op=OP.max)
                nc.vector.tensor_tensor(out=x[:, 0:2, :], in0=x[:, 0:2, :],
                                        in1=x[:, 2:4, :], op=OP.max)
                rst = wp.tile([P, D], BF16, tag="rst")
                nc.vector.tensor_tensor(out=rst[:], in0=x[:, 0, :],
                                        in1=x[:, 1, :], op=OP.max)
                rT = wp.tile([P, DC, P], BF16, tag="asb")
                for c in range(DC):
                    nc.sync.dma_start_transpose(rT[:, c, :],
                                                rst[:, c * P:(c + 1) * P])
                pc = psB.tile([P, D], F32, tag="mm")
                for c in range(DC):
                    nc.tensor.matmul(pc[:, 0:NCLS], rT[:, c, :], wf_sb[:, c, :],
                                     start=(c == 0), stop=(c == DC - 1))
                if flags['bfc']:
                    nc.vector.tensor_add(pc[:, 0:NCLS], pc[:, 0:NCLS],
                                         vec_sb['bfc'][:, :])
                lg = wp.tile([P, NCLS], F32, tag="lg")
                nc.scalar.copy(lg[:], pc[:, 0:NCLS])
                nc.sync.dma_start(out_d[ds(i * P, P), :], lg[:])

            # ---- software-pipelined emission ----
            st = {}
            emit_gather(0, st)
            emit_A(0, 0, st)
            if NT > 1:
                emit_gather(1, st)
                emit_A(1, 0, st)
            emit_Bq(0, 0, st)
            emit_Bv(0, 0, st)
            emit_C(0, 0, st)
            for i in range(NT):
                emit_A(i, 1, st)
                if i + 2 < NT:
                    emit_gather(i + 2, st)
                    emit_A_mm(i + 2, 0, st)
                if i + 1 < NT:
                    emit_Bq(i + 1, 0, st)
                    emit_Bv(i + 1, 0, st)
                emit_Bq(i, 1, st)
                emit_Bv(i, 1, st)
                if i + 2 < NT:
                    emit_A_cp(i + 2, 0, st)
                if i + 1 < NT:
                    emit_C(i + 1, 0, st)
                emit_C(i, 1, st)
                emit_epi(i, st)

    _split_multiwait_drains(nc)
    return nc


OPT_KEYS = ('bqkv', 'bo', 'b1', 'b2', 'bfc', 'ln_g', 'ln_b')
_cache = {}


def _get_nc(flags):
    key = tuple(flags[k] for k in OPT_KEYS)
    if key not in _cache:
        _cache[key] = build(flags)
    return _cache[key]


def _prep_inputs(inputs, flags):
    bf = ml_dtypes.bfloat16
    token_ids = np.asarray(inputs['token_ids'])
    edge_src = np.asarray(inputs['edge_src'])
    emb = np.asarray(inputs['emb'], dtype=np.float32)
    Wqkv = np.asarray(inputs['Wqkv'], dtype=np.float32)
    Wo = np.asarray(inputs['Wo'], dtype=np.float32)
    W1 = np.asarray(inputs['W1'], dtype=np.float32)
    W2 = np.asarray(inputs['W2'], dtype=np.float32)
    Wfc = np.asarray(inputs['Wfc'], dtype=np.float32)

    tid2 = token_ids[edge_src[:, :S]]                      # [NDST, S]
    xg = emb.astype(bf)[tid2]                              # [NDST, S, D]
    wqkvT = np.ascontiguousarray(Wqkv.transpose(0, 2, 1))  # [NL, D, 3D]
    wqkvT[:, :, 0:D] *= 0.125                              # fold q scale
    common = {
        'wqkvT': wqkvT.astype(bf),
        'woT': np.ascontiguousarray(Wo.transpose(0, 2, 1)).astype(bf),
        'w1T': np.ascontiguousarray(W1.transpose(0, 2, 1)).astype(bf),
        'w2T': np.ascontiguousarray(W2.transpose(0, 2, 1)).astype(bf),
        'wfcT': np.ascontiguousarray(Wfc.T).astype(bf),
        'ident': np.eye(P, dtype=bf),
    }
    if flags['bqkv']:
        bq = np.asarray(inputs['bqkv'], dtype=np.float32).copy()
        bq[:, 0:D] *= 0.125
        common['bqkv'] = np.ascontiguousarray(
            np.broadcast_to(bq.reshape(1, -1), (P, NL * 3 * D)))
    if flags['bo']:
        common['bo'] = np.ascontiguousarray(np.broadcast_to(
            np.asarray(inputs['bo'], np.float32).reshape(1, -1),
            (P, NL * D)))
    if flags['b1']:
        b1 = np.asarray(inputs['b1'], dtype=np.float32)
        common['b1t'] = np.ascontiguousarray(
            b1.reshape(NL, FCH, P).transpose(2, 0, 1).reshape(P, NL * FCH))
    if flags['b2']:
        common['b2'] = np.ascontiguousarray(np.broadcast_to(
            np.asarray(inputs['b2'], np.float32).reshape(1, -1),
            (P, NL * D)))
    if flags['bfc']:
        common['bfc'] = np.ascontiguousarray(np.broadcast_to(
            np.asarray(inputs['bfc'], np.float32).reshape(1, -1),
            (P, NCLS)))
    if flags['ln_g']:
        common['ln1_g'] = np.ascontiguousarray(np.broadcast_to(
            np.asarray(inputs['ln1_g'], np.float32).reshape(1, -1),
            (P, NL * D)))
        common['ln2_g'] = np.ascontiguousarray(np.broadcast_to(
            np.asarray(inputs['ln2_g'], np.float32).reshape(1, -1),
            (P, NL * D)))
    if flags['ln_b']:
        common['ln1_b'] = np.ascontiguousarray(np.broadcast_to(
            np.asarray(inputs['ln1_b'], np.float32).reshape(1, -1),
            (P, NL * D)))
        common['ln2_b'] = np.ascontiguousarray(np.broadcast_to(
            np.asarray(inputs['ln2_b'], np.float32).reshape(1, -1),
            (P, NL * D)))

    in_maps = []
    for c in range(NCORES):
        m = dict(common)
        xc = xg[c * NLOC:(c + 1) * NLOC]                   # [NLOC, S, D]
        m['xg'] = np.ascontiguousarray(xc.reshape(NLOC, S * D))
        # xt0[i, c, p, s*128+n] = x[i*128+n, s, c*128+p]
        m['xt0'] = np.ascontiguousarray(
            xc.reshape(NT, P, S, DC, P).transpose(0, 3, 4, 2, 1)
            .reshape(NT, DC, P, S * P))
        in_maps.append(m)
    return in_maps


def _get_flags(inputs):
    return {
        'bqkv': bool(np.any(inputs['bqkv'])),
        'bo': bool(np.any(inputs['bo'])),
        'b1': bool(np.any(inputs['b1'])),
        'b2': bool(np.any(inputs['b2'])),
        'bfc': bool(np.any(inputs['bfc'])),
        'ln_g': bool(np.any(np.asarray(inputs['ln1_g']) != 1.0)
                     or np.any(np.asarray(inputs['ln2_g']) != 1.0)),
        'ln_b': bool(np.any(inputs['ln1_b']) or np.any(inputs['ln2_b'])),
    }


def kernel(**inputs):
    flags = _get_flags(inputs)
    nc = _get_nc(flags)
    in_maps = _prep_inputs(inputs, flags)
    res = run_bass_kernel_spmd(nc, in_maps, core_ids=list(range(NCORES)))
    out = np.concatenate([res.results[c]['logits'] for c in range(NCORES)],
                         axis=0)
    return out.astype(np.float32)


if __name__ == '__main__':
    import time
    sys.path.insert(0, '/root/problem')
    import reference
    inp = {k: np.asarray(v) for k, v in reference.setup_inputs().items()}
    t0 = time.time()
    got = kernel(**inp)
    print(f"kernel ran in {time.time()-t0:.1f}s")
    exp = np.asarray(reference.reference(**reference.setup_inputs()))
    err = np.abs(got - exp).max()
    rel = err / np.abs(exp).max()
    print(f"absmax err {err:.3e}  rel {rel:.3e}")
